# revision 46
# baseline (speedup 1.0000x reference)
"""KoLeo loss kernel for 8 Trainium2 NeuronCores — symmetric (half-matrix)
variant.

Reference computation (B=16384, D=1024):
    xn  = x / max(||x||_2, 1e-12)          # row L2-normalize
    sim = xn @ xn.T                        # B x B cosine similarity
    max_sim[i] = max_{j != i} sim[i, j]    # nearest neighbor (excl. self)
    out = -mean(log(sqrt(2 - 2*max_sim + 1e-8)))

sim is symmetric, so only the upper triangle of 512x512 super-blocks is
computed (~53% of the matmul work of the dense baseline). Each computed
super-block (I, J), I < J, serves rows of I via its row-max and rows of J
via its column-max. Per-super-block epilogue (every stage fits under the
Tensor engine's ~3.6 us of matmul work; Pool cannot touch PSUM on TRN2
and its partition_all_reduce is too slow at ~3.4 ns/output element, so
Pool stays idle — fewer busy engines also eases the chip power throttle):
  - ACT: sole bulk PSUM consumer — one [128, 2048] copy stages the whole
    4-bank tile to fp16 SBUF (~2.5 us: 1 elem/cycle + ~0.7 us fixed), so
    the PE recycles its two psum buffers quickly.
  - DVE: row-max reduce of the staged tile (reduce has no 16-bit fast
    mode, ~2.2 us), then two fp16 2x tensor_max folds to [128, 512].
  - DMA: ships the folded tile to DRAM (~128KB/block).
  - Host: 128-partition max + scatter into the global row maxima.

Work distribution ("pencil window", SPMD-uniform): global super-row G is
owned by core c = G % 8. Each core holds its 4 owned super-rows resident
(local positions 8a after a per-core rotation of x.T columns by 512*c) and
computes super-blocks (I, I+w mod 32) for w = 0..16 (a in {0,1}) or
w = 0..15 (a in {2,3}). Every unordered pair of super-blocks is covered
exactly once across the fleet (528 total); w=0 is the diagonal block,
where a -2*scale^2*I constant is added at the self-similarity positions
before the row max so the self-match never wins. Per core: 66
super-blocks = 1056 fp8 DoubleRow matmuls (~229 us at the 157 TF/s fp8
roofline, vs 437 us for the dense baseline).

The input is laid out host-side as [J, p, k, j] (so each 512-column block
is one contiguous [128 partitions x 4KB] slab) and each rhs block loads
with a single DMA — ~130 DMA issues total instead of ~530.

Host: pre-normalizes rows (f64), scales by 8 and casts to fp8e4m3,
pre-rotates/retiles per core; post-merges row/column maxima across cores
and applies the scalar log epilogue in f64.
"""

import sys

if "/opt/trn_rl_repo" not in sys.path:
    sys.path.insert(0, "/opt/trn_rl_repo")

import numpy as np
import ml_dtypes

import concourse.bass as bass  # noqa: F401  (import keeps bass registered)
import concourse.mybir as mybir
import concourse.tile as tile
from concourse import bacc
from concourse.bass_utils import run_bass_kernel_spmd

P = 128          # SBUF partitions
NBLK = 512       # super-block side (= one PSUM bank of f32 per 128 rows)
EPS = 1e-8

B = 16384        # rows of x
D = 1024         # feature dim
N_CORES = 8
NSB = B // NBLK  # 32 super-blocks per matrix side
KCH = D // P     # 8 contraction chunks of 128
KSTEP = 2        # fp8 DoubleRow: K chunks of 256 per matmul
NA = 4           # owned super-rows per core (global stride 8)
FP8_SCALE = 8.0


def _windows():
    """Program-order (a, w) list. w=0 is the diagonal super-block.

    a in {0,1} get w up to 16, a in {2,3} up to 15: the distance-16 pairs
    {i, i+16} are covered once by the a0/a1 windows (i = c + 8*a0), so the
    a2/a3 windows stop at 15. Total 66 super-blocks per core; the union of
    (owned I, I+w) over all cores covers every unordered block pair once.
    """
    out = []
    for a in range(NA):
        wmax = 16 if a < 2 else 15
        ws = list(range(wmax + 1))
        if a > 0:
            # diagonal (w=0) last: its epilogue has no colmax leg, which
            # shortens the critical tail after the final matmul. a=0 keeps
            # the diagonal first — it runs from the resident lhs tile while
            # the first rhs DMAs are still in flight.
            ws = ws[1:] + [0]
        for w in ws:
            out.append((a, w))
    return out


def _upper_order():
    """Program-order list of the 62 strictly-upper (a, w) super-blocks."""
    return [(a, w) for a, w in _windows() if w > 0]


N_UP = len(_upper_order())  # 62


def build_nc():
    """Build the per-core SPMD Bass program.

    Inputs :  xt     [NSB*P, KCH*NBLK] fp8e4m3 — normalized, scaled x.T,
              retiled as [J, p, k, j] and column-rotated by 512*c so owned
              super-rows sit at local block 8a.
              negeye [P, P] f32 — the constant -2*scale^2 * I
    Outputs:  rowmax [P, 16]         f32 — [p, 4a+r] = row-max over the
              computed window for local row 4096a + 128r + p
              colmax [N_UP, 2*NBLK]  f16 — per strictly-upper super-block
              (program order), the 128-partition max of the pair-folded
              chunks: col j holds max(chunk0,chunk1) over partitions,
              col NBLK+j holds max(chunk2,chunk3); host folds the pair.
    """
    f32 = mybir.dt.float32
    f16 = mybir.dt.float16
    fp8 = mybir.dt.float8e4
    ngrp = KCH // KSTEP

    nc = bacc.Bacc("TRN2", target_bir_lowering=False, debug=False,
                   num_devices=N_CORES)
    xt = nc.dram_tensor("xt", [NSB * P, KCH * NBLK], fp8,
                        kind="ExternalInput")
    negeye = nc.dram_tensor("negeye", [P, P], f32, kind="ExternalInput")
    rowmax_d = nc.dram_tensor("rowmax", [P, NA * 4], f32,
                              kind="ExternalOutput")
    colmax_d = nc.dram_tensor("colmax", [N_UP * P, NBLK], f16,
                              kind="ExternalOutput")
    xt_ap = xt[:]
    colmax_ap = colmax_d[:]

    with tile.TileContext(nc) as tc:
        with (
            tc.tile_pool(name="lhs", bufs=1) as lhs_pool,
            tc.tile_pool(name="rhs", bufs=3) as rhs_pool,
            tc.tile_pool(name="psum", bufs=2, space="PSUM") as psum_pool,
            tc.tile_pool(name="stage", bufs=3) as stage_pool,
            tc.tile_pool(name="stats", bufs=1) as stats_pool,
        ):
            dma_eng = [nc.sync, nc.scalar]
            ndma = 0

            lhs_tiles = []
            for a in range(NA):
                t = lhs_pool.tile([P, KCH, NBLK], fp8, name=f"lhs{a}",
                                  tag=f"lhs{a}")
                lhs_tiles.append(t)
                rows = slice(8 * a * P, (8 * a + 1) * P)
                if a == 0:
                    # split across both queues so the first matmul's
                    # operand lands ~1 us earlier
                    half = KCH // 2
                    nc.sync.dma_start(t[:, 0:half, :],
                                      xt_ap[rows, 0:half * NBLK])
                    nc.scalar.dma_start(t[:, half:, :],
                                        xt_ap[rows, half * NBLK:])
                else:
                    dma_eng[ndma % 2].dma_start(t[:], xt_ap[rows, :])
                    ndma += 1
            eye = stats_pool.tile([P, P], f32, name="eye")
            nc.sync.dma_start(eye[:], negeye[:])
            maxt = [
                stats_pool.tile([P, 4, 17], f32, name=f"maxt{a}",
                                tag=f"maxt{a}")
                for a in range(NA)
            ]
            rowmax_sb = stats_pool.tile([P, NA * 4], f32, name="rowmax_sb")

            wins = _windows()
            last_w_of_a = {a: w for a, w in wins}  # last (a, w) in order
            sb_idx = 0  # strictly-upper super-block output row
            for a, w in wins:
                J = (8 * a + w) % NSB
                if w == 0:
                    rt = None  # rhs block is the resident lhs tile itself
                else:
                    rt = rhs_pool.tile([P, KCH, NBLK], fp8, name="rt",
                                       tag="rt")
                    dma_eng[ndma % 2].dma_start(
                        rt[:], xt_ap[J * P:(J + 1) * P, :])
                    ndma += 1

                # One 4-bank psum tile per super-block (chunks r = 0..3).
                ps = psum_pool.tile([P, 4, NBLK], f32, name="ps", tag="ps")
                for r in range(4):
                    for g in range(ngrp):
                        ks = slice(KSTEP * g, KSTEP * (g + 1))
                        rhs = (lhs_tiles[a][:, ks, :] if rt is None
                               else rt[:, ks, :])
                        nc.tensor.matmul(
                            ps[:, r, :],
                            lhs_tiles[a][:, ks, r * P:(r + 1) * P],
                            rhs,
                            start=(g == 0),
                            stop=(g == ngrp - 1),
                            perf_mode=mybir.MatmulPerfMode.DoubleRow,
                        )

                if w == 0:
                    # self-similarity of chunk r lives at [p, r*P + p]:
                    # add -2*scale^2*I so the self-match never wins.
                    for r in range(4):
                        sl = ps[:, r, r * P:(r + 1) * P]
                        nc.vector.tensor_add(out=sl, in0=sl, in1=eye[:])

                # ACT is the sole bulk PSUM consumer: one [128, 2048] copy
                # stages the whole tile to fp16 SBUF (~2.5 us), so the PE
                # can recycle the psum buffer quickly. DVE reduces the
                # staged copy instead of PSUM.
                st = stage_pool.tile([P, 4, NBLK], f16, name="st", tag="st")
                nc.scalar.copy(st[:], ps[:])

                nc.vector.reduce_max(
                    out=maxt[a][:, :, w:w + 1],
                    in_=st[:],
                    axis=mybir.AxisListType.X,
                    op=mybir.AluOpType.max,
                )

                if w > 0:
                    stm = stage_pool.tile([P, 2, NBLK], f16, name="stm",
                                          tag="stm", bufs=4)
                    nc.vector.tensor_max(out=stm[:], in0=st[:, 0:2, :],
                                         in1=st[:, 2:4, :])
                    mg = stage_pool.tile([P, NBLK], f16, name="mg",
                                         tag="mg", bufs=6)
                    nc.vector.tensor_max(out=mg[:], in0=stm[:, 0, :],
                                         in1=stm[:, 1, :])
                    # Ship the folded [128, 512] tile; the host does the
                    # 128-partition max — cheaper than Pool's
                    # partition_all_reduce (~3.9 us/block on the Q7s).
                    dma_eng[ndma % 2].dma_start(
                        colmax_ap[sb_idx * P:(sb_idx + 1) * P, :], mg[:])
                    ndma += 1
                    sb_idx += 1

                if w == last_w_of_a[a]:
                    # fold this super-row's row maxima as soon as its last
                    # window lands, keeping only a=3's fold in the tail
                    nw = 17 if a < 2 else 16
                    nc.vector.reduce_max(
                        out=rowmax_sb[:, 4 * a:4 * a + 4],
                        in_=maxt[a][:, :, 0:nw],
                        axis=mybir.AxisListType.X,
                        op=mybir.AluOpType.max,
                    )
            nc.sync.dma_start(rowmax_d[:], rowmax_sb[:])

    nc.compile()
    return nc


def prepare_inputs(x):
    """Host prep: normalize (f64), scale+cast fp8, retile, rotate."""
    xd = np.asarray(x, dtype=np.float64)
    norms = np.sqrt(np.einsum("ij,ij->i", xd, xd))
    np.maximum(norms, 1e-12, out=norms)
    xn = xd / norms[:, None]
    xnt = (xn.T * FP8_SCALE).astype(ml_dtypes.float8_e4m3)  # [D, B]
    # retile to [J, p, k, j]: xt_r[J, p, k, j] = xnt[k*128+p, J*512+j]
    xt_r = np.ascontiguousarray(
        xnt.reshape(KCH, P, NSB, NBLK).transpose(2, 1, 0, 3))
    negeye = np.ascontiguousarray(
        (-2.0 * FP8_SCALE * FP8_SCALE) * np.eye(P, dtype=np.float32))
    in_maps = []
    for c in range(N_CORES):
        rot = (np.concatenate([xt_r[c:], xt_r[:c]], axis=0) if c
               else xt_r)
        in_maps.append({
            "xt": np.ascontiguousarray(rot).reshape(NSB * P, KCH * NBLK),
            "negeye": negeye,
        })
    return in_maps


def postprocess(results):
    """Merge per-core row/column maxima and apply the scalar epilogue."""
    inv = 1.0 / (FP8_SCALE * FP8_SCALE)
    order = _upper_order()
    maxsim = np.full(B, -np.inf, dtype=np.float64)
    for c in range(N_CORES):
        rm = np.asarray(results[c]["rowmax"], dtype=np.float64)  # [P, 16]
        for a in range(NA):
            for r in range(4):
                g0 = (c + 8 * a) * NBLK + r * P  # global row of partition 0
                sl = slice(g0, g0 + P)
                np.maximum(maxsim[sl], rm[:, 4 * a + r], out=maxsim[sl])
        cmx = np.asarray(results[c]["colmax"]).astype(np.float32)
        # fold the 128 partitions on the host
        cmx = cmx.reshape(N_UP, P, NBLK).max(axis=1).astype(np.float64)
        for s, (a, w) in enumerate(order):
            g0 = ((8 * a + w + c) % NSB) * NBLK
            sl = slice(g0, g0 + NBLK)
            np.maximum(maxsim[sl], cmx[s], out=maxsim[sl])
    d2 = 2.0 - 2.0 * (maxsim * inv) + EPS
    loss = -0.5 * np.mean(np.log(d2))
    return np.array(loss, dtype=np.float32)


_NC_CACHE = {}


def _get_nc():
    if "nc" not in _NC_CACHE:
        _NC_CACHE["nc"] = build_nc()
    return _NC_CACHE["nc"]


def kernel(x, **_ignored):
    import time

    nc = _get_nc()
    in_maps = prepare_inputs(x)
    last_exc = None
    for attempt in range(3):
        try:
            res = run_bass_kernel_spmd(nc, in_maps,
                                       core_ids=list(range(N_CORES)))
            return postprocess(res.results)
        except Exception as exc:  # transient NRT/tunnel hiccups
            last_exc = exc
            if attempt < 2:
                time.sleep(30)  # a wedged exec unit takes a while to heal
    raise last_exc


if __name__ == "__main__":
    x = np.random.default_rng(0).standard_normal((B, D), dtype=np.float32)
    print(kernel(x))


# revision 50
# speedup vs baseline: 1.1464x; 1.1464x over previous
"""KoLeo loss kernel for 8 Trainium2 NeuronCores — symmetric (half-matrix)
variant.

Reference computation (B=16384, D=1024):
    xn  = x / max(||x||_2, 1e-12)          # row L2-normalize
    sim = xn @ xn.T                        # B x B cosine similarity
    max_sim[i] = max_{j != i} sim[i, j]    # nearest neighbor (excl. self)
    out = -mean(log(sqrt(2 - 2*max_sim + 1e-8)))

sim is symmetric, so only the upper triangle of 512x512 super-blocks is
computed (~53% of the matmul work of the dense baseline). Each computed
super-block (I, J), I < J, serves rows of I via its row-max and rows of J
via its column-max. Per-super-block epilogue (every stage fits under the
Tensor engine's ~3.6 us of matmul work; Pool cannot touch PSUM on TRN2
and its partition_all_reduce is too slow at ~3.4 ns/output element, so
Pool stays idle — fewer busy engines also eases the chip power throttle):
  - ACT: sole bulk PSUM consumer — one [128, 2048] copy stages the whole
    4-bank tile to fp16 SBUF (~2.5 us: 1 elem/cycle + ~0.7 us fixed), so
    the PE recycles its two psum buffers quickly.
  - DVE: row-max reduce of the staged tile (reduce has no 16-bit fast
    mode, ~2.2 us), then two fp16 2x tensor_max folds to [128, 512].
  - DMA: ships the folded tile to DRAM (~128KB/block).
  - Host: 128-partition max + scatter into the global row maxima.

Work distribution ("pencil window", SPMD-uniform): global super-row G is
owned by core c = G % 8. Each core holds its 4 owned super-rows resident
(local positions 8a after a per-core rotation of x.T columns by 512*c) and
computes super-blocks (I, I+w mod 32) for w = 0..16 (a in {0,1}) or
w = 0..15 (a in {2,3}). Every unordered pair of super-blocks is covered
exactly once across the fleet (528 total); w=0 is the diagonal block,
where a -2*scale^2*I constant is added at the self-similarity positions
before the row max so the self-match never wins. Per core: 66
super-blocks = 1056 fp8 DoubleRow matmuls (~229 us at the 157 TF/s fp8
roofline, vs 437 us for the dense baseline).

The input is laid out host-side as [J, p, k, j] (so each 512-column block
is one contiguous [128 partitions x 4KB] slab) and each rhs block loads
with a single DMA — ~130 DMA issues total instead of ~530.

Host: pre-normalizes rows (f64), scales by 8 and casts to fp8e4m3,
pre-rotates/retiles per core; post-merges row/column maxima across cores
and applies the scalar log epilogue in f64.
"""

import sys

if "/opt/trn_rl_repo" not in sys.path:
    sys.path.insert(0, "/opt/trn_rl_repo")

import numpy as np
import ml_dtypes

import concourse.bass as bass  # noqa: F401  (import keeps bass registered)
import concourse.mybir as mybir
import concourse.tile as tile
from concourse import bacc
from concourse.bass_utils import run_bass_kernel_spmd

P = 128          # SBUF partitions
NBLK = 512       # super-block side (= one PSUM bank of f32 per 128 rows)
EPS = 1e-8

B = 16384        # rows of x
D = 1024         # feature dim
N_CORES = 8
NSB = B // NBLK  # 32 super-blocks per matrix side
KCH = D // P     # 8 contraction chunks of 128
KSTEP = 2        # fp8 DoubleRow: K chunks of 256 per matmul
NA = 4           # owned super-rows per core (global stride 8)
FP8_SCALE = 8.0


def _windows():
    """Program-order (a, w) list. w=0 is the diagonal super-block.

    a in {0,1} get w up to 16, a in {2,3} up to 15: the distance-16 pairs
    {i, i+16} are covered once by the a0/a1 windows (i = c + 8*a0), so the
    a2/a3 windows stop at 15. Total 66 super-blocks per core; the union of
    (owned I, I+w) over all cores covers every unordered block pair once.
    """
    out = []
    for a in range(NA):
        wmax = 16 if a < 2 else 15
        for w in range(wmax + 1):
            out.append((a, w))
    return out


def _upper_order():
    """Program-order list of the 62 strictly-upper (a, w) super-blocks."""
    return [(a, w) for a, w in _windows() if w > 0]


N_UP = len(_upper_order())  # 62


def build_nc():
    """Build the per-core SPMD Bass program.

    Inputs :  xt     [NSB*P, KCH*NBLK] fp8e4m3 — normalized, scaled x.T,
              retiled as [J, p, k, j] and column-rotated by 512*c so owned
              super-rows sit at local block 8a.
              negeye [P, P] f32 — the constant -2*scale^2 * I
    Outputs:  rowmax [P, 16]         f32 — [p, 4a+r] = row-max over the
              computed window for local row 4096a + 128r + p
              colmax [N_UP, 2*NBLK]  f16 — per strictly-upper super-block
              (program order), the 128-partition max of the pair-folded
              chunks: col j holds max(chunk0,chunk1) over partitions,
              col NBLK+j holds max(chunk2,chunk3); host folds the pair.
    """
    f32 = mybir.dt.float32
    f16 = mybir.dt.float16
    fp8 = mybir.dt.float8e4
    ngrp = KCH // KSTEP

    nc = bacc.Bacc("TRN2", target_bir_lowering=False, debug=False,
                   num_devices=N_CORES)
    xt = nc.dram_tensor("xt", [NSB * P, KCH * NBLK], fp8,
                        kind="ExternalInput")
    negeye = nc.dram_tensor("negeye", [P, P], f32, kind="ExternalInput")
    rowmax_d = nc.dram_tensor("rowmax", [P, NA * 4], f32,
                              kind="ExternalOutput")
    colmax_d = nc.dram_tensor("colmax", [N_UP * P, NBLK], f16,
                              kind="ExternalOutput")
    xt_ap = xt[:]
    colmax_ap = colmax_d[:]

    with tile.TileContext(nc) as tc:
        with (
            tc.tile_pool(name="lhs", bufs=1) as lhs_pool,
            tc.tile_pool(name="rhs", bufs=3) as rhs_pool,
            tc.tile_pool(name="psum", bufs=2, space="PSUM") as psum_pool,
            tc.tile_pool(name="stage", bufs=3) as stage_pool,
            tc.tile_pool(name="stats", bufs=1) as stats_pool,
        ):
            dma_eng = [nc.sync, nc.scalar]
            ndma = 0

            lhs_tiles = []
            for a in range(NA):
                t = lhs_pool.tile([P, KCH, NBLK], fp8, name=f"lhs{a}",
                                  tag=f"lhs{a}")
                lhs_tiles.append(t)
                rows = slice(8 * a * P, (8 * a + 1) * P)
                dma_eng[ndma % 2].dma_start(t[:], xt_ap[rows, :])
                ndma += 1
            eye = stats_pool.tile([P, P], f32, name="eye")
            nc.sync.dma_start(eye[:], negeye[:])
            maxt = [
                stats_pool.tile([P, 4, 17], f32, name=f"maxt{a}",
                                tag=f"maxt{a}")
                for a in range(NA)
            ]
            rowmax_sb = stats_pool.tile([P, NA * 4], f32, name="rowmax_sb")

            sb_idx = 0  # strictly-upper super-block output row
            for a, w in _windows():
                J = (8 * a + w) % NSB
                if w == 0:
                    rt = None  # rhs block is the resident lhs tile itself
                else:
                    rt = rhs_pool.tile([P, KCH, NBLK], fp8, name="rt",
                                       tag="rt")
                    dma_eng[ndma % 2].dma_start(
                        rt[:], xt_ap[J * P:(J + 1) * P, :])
                    ndma += 1

                # One 4-bank psum tile per super-block (chunks r = 0..3).
                ps = psum_pool.tile([P, 4, NBLK], f32, name="ps", tag="ps")
                for r in range(4):
                    for g in range(ngrp):
                        ks = slice(KSTEP * g, KSTEP * (g + 1))
                        rhs = (lhs_tiles[a][:, ks, :] if rt is None
                               else rt[:, ks, :])
                        nc.tensor.matmul(
                            ps[:, r, :],
                            lhs_tiles[a][:, ks, r * P:(r + 1) * P],
                            rhs,
                            start=(g == 0),
                            stop=(g == ngrp - 1),
                            perf_mode=mybir.MatmulPerfMode.DoubleRow,
                        )

                if w == 0:
                    # self-similarity of chunk r lives at [p, r*P + p]:
                    # add -2*scale^2*I so the self-match never wins.
                    for r in range(4):
                        sl = ps[:, r, r * P:(r + 1) * P]
                        nc.vector.tensor_add(out=sl, in0=sl, in1=eye[:])

                # ACT is the sole bulk PSUM consumer: one [128, 2048] copy
                # stages the whole tile to fp16 SBUF (~2.5 us), so the PE
                # can recycle the psum buffer quickly. DVE reduces the
                # staged copy instead of PSUM.
                st = stage_pool.tile([P, 4, NBLK], f16, name="st", tag="st")
                nc.scalar.copy(st[:], ps[:])

                nc.vector.reduce_max(
                    out=maxt[a][:, :, w:w + 1],
                    in_=st[:],
                    axis=mybir.AxisListType.X,
                    op=mybir.AluOpType.max,
                )

                if w > 0:
                    stm = stage_pool.tile([P, 2, NBLK], f16, name="stm",
                                          tag="stm", bufs=4)
                    nc.vector.tensor_max(out=stm[:], in0=st[:, 0:2, :],
                                         in1=st[:, 2:4, :])
                    mg = stage_pool.tile([P, NBLK], f16, name="mg",
                                         tag="mg", bufs=6)
                    nc.vector.tensor_max(out=mg[:], in0=stm[:, 0, :],
                                         in1=stm[:, 1, :])
                    # Ship the folded [128, 512] tile; the host does the
                    # 128-partition max — cheaper than Pool's
                    # partition_all_reduce (~3.9 us/block on the Q7s).
                    dma_eng[ndma % 2].dma_start(
                        colmax_ap[sb_idx * P:(sb_idx + 1) * P, :], mg[:])
                    ndma += 1
                    sb_idx += 1

            for a in range(NA):
                nw = 17 if a < 2 else 16
                nc.vector.reduce_max(
                    out=rowmax_sb[:, 4 * a:4 * a + 4],
                    in_=maxt[a][:, :, 0:nw],
                    axis=mybir.AxisListType.X,
                    op=mybir.AluOpType.max,
                )
            nc.sync.dma_start(rowmax_d[:], rowmax_sb[:])

    nc.compile()
    return nc


def prepare_inputs(x):
    """Host prep: normalize (f64), scale+cast fp8, retile, rotate."""
    xd = np.asarray(x, dtype=np.float64)
    norms = np.sqrt(np.einsum("ij,ij->i", xd, xd))
    np.maximum(norms, 1e-12, out=norms)
    xn = xd / norms[:, None]
    xnt = (xn.T * FP8_SCALE).astype(ml_dtypes.float8_e4m3)  # [D, B]
    # retile to [J, p, k, j]: xt_r[J, p, k, j] = xnt[k*128+p, J*512+j]
    xt_r = np.ascontiguousarray(
        xnt.reshape(KCH, P, NSB, NBLK).transpose(2, 1, 0, 3))
    negeye = np.ascontiguousarray(
        (-2.0 * FP8_SCALE * FP8_SCALE) * np.eye(P, dtype=np.float32))
    in_maps = []
    for c in range(N_CORES):
        rot = (np.concatenate([xt_r[c:], xt_r[:c]], axis=0) if c
               else xt_r)
        in_maps.append({
            "xt": np.ascontiguousarray(rot).reshape(NSB * P, KCH * NBLK),
            "negeye": negeye,
        })
    return in_maps


def postprocess(results):
    """Merge per-core row/column maxima and apply the scalar epilogue."""
    inv = 1.0 / (FP8_SCALE * FP8_SCALE)
    order = _upper_order()
    maxsim = np.full(B, -np.inf, dtype=np.float64)
    for c in range(N_CORES):
        rm = np.asarray(results[c]["rowmax"], dtype=np.float64)  # [P, 16]
        for a in range(NA):
            for r in range(4):
                g0 = (c + 8 * a) * NBLK + r * P  # global row of partition 0
                sl = slice(g0, g0 + P)
                np.maximum(maxsim[sl], rm[:, 4 * a + r], out=maxsim[sl])
        cmx = np.asarray(results[c]["colmax"]).astype(np.float32)
        # fold the 128 partitions on the host
        cmx = cmx.reshape(N_UP, P, NBLK).max(axis=1).astype(np.float64)
        for s, (a, w) in enumerate(order):
            g0 = ((8 * a + w + c) % NSB) * NBLK
            sl = slice(g0, g0 + NBLK)
            np.maximum(maxsim[sl], cmx[s], out=maxsim[sl])
    d2 = 2.0 - 2.0 * (maxsim * inv) + EPS
    loss = -0.5 * np.mean(np.log(d2))
    return np.array(loss, dtype=np.float32)


_NC_CACHE = {}


def _get_nc():
    if "nc" not in _NC_CACHE:
        _NC_CACHE["nc"] = build_nc()
    return _NC_CACHE["nc"]


def kernel(x, **_ignored):
    import time

    nc = _get_nc()
    in_maps = prepare_inputs(x)
    last_exc = None
    for attempt in range(3):
        try:
            res = run_bass_kernel_spmd(nc, in_maps,
                                       core_ids=list(range(N_CORES)))
            return postprocess(res.results)
        except Exception as exc:  # transient NRT/tunnel hiccups
            last_exc = exc
            if attempt < 2:
                time.sleep(30)  # a wedged exec unit takes a while to heal
    raise last_exc


if __name__ == "__main__":
    x = np.random.default_rng(0).standard_normal((B, D), dtype=np.float32)
    print(kernel(x))
